# revision 4
# baseline (speedup 1.0000x reference)
"""Trainium2 Bass kernel for AnnealingTopKSoftMax (top-8 masked softmax).

Computes, for each row of a [131072, 512] f32 tensor:
  out = softmax(where(mask_top8(x), x, -1e16))
which equals: exp(x) / sum(exp(top8(x))) at the top-8 positions, 0 elsewhere.

Strategy (pure data parallelism, batch axis sharded over 8 NeuronCores):
  per [128, 512] tile (rows on partitions):
    v8 = max8(x)            # DVE: 8 largest per row, descending
    e  = exp(x)             # ACT (no max-subtraction needed: |x| <= ~6)
    e8, s = exp(v8), sum    # ACT tiny, accum_out gives denominator (8 terms)
    z  = match_replace(e, needles=exp(v8), 0)   # DVE: zero exactly the top-8
    d  = e - z              # DVE: keep ONLY the top-8 exps
    out = d * (1/s)         # GpSimd: per-row scale
match_replace replaces exactly one occurrence per needle (first match), which
reproduces jax.lax.top_k's lowest-index tie-breaking.
"""

import os
import sys
import types

import numpy as np

import concourse.bacc as bacc
import concourse.tile as tile
from concourse import mybir
from concourse.bass_utils import run_bass_kernel_spmd


def _install_ntff_hook() -> bool:
    """Provide antenv.axon_hooks (absent in this container) so
    run_bass_kernel_spmd(trace=True) can capture NTFF profiles under axon.
    Mirrors trn_agent_boot's registration. Returns False when unavailable."""
    try:
        from antenv.axon_hooks import get_axon_ntff_profile_hook  # noqa: F401

        return True
    except ImportError:
        pass
    try:
        import antenv
        from trn_agent_boot.trn_boot import _ntff_profile_via_ctypes

        hook = _ntff_profile_via_ctypes("/opt/axon/libaxon_pjrt.so")
        mod = types.ModuleType("antenv.axon_hooks")
        _h = [hook]
        mod.set_axon_ntff_profile_hook = lambda h: _h.__setitem__(0, h)
        mod.get_axon_ntff_profile_hook = lambda: _h[0]
        sys.modules["antenv.axon_hooks"] = mod
        antenv.axon_hooks = mod
        return hook is not None
    except Exception:
        return False

N_CORES = 8
BATCH = 131072
DEPTH = 512
ROWS_PER_CORE = BATCH // N_CORES  # 16384
P = 128          # SBUF partitions; rows per sub-tile
C = 8            # consecutive rows per partition per block (16KB contiguous DMA)
BLOCK_ROWS = P * C               # 1024
N_BLOCKS = ROWS_PER_CORE // BLOCK_ROWS  # 16

F32 = mybir.dt.float32
Exp = mybir.ActivationFunctionType.Exp


def _build():
    nc = bacc.Bacc(
        "TRN2", target_bir_lowering=False, debug=False, num_devices=N_CORES
    )
    x = nc.dram_tensor("x", [ROWS_PER_CORE, DEPTH], F32, kind="ExternalInput")
    out = nc.dram_tensor("out", [ROWS_PER_CORE, DEPTH], F32, kind="ExternalOutput")

    # row = n*1024 + p*8 + c  ->  partition p holds 8 consecutive rows per block
    xv = x.ap().rearrange("(n p c) d -> p n c d", p=P, c=C)
    ov = out.ap().rearrange("(n p c) d -> p n c d", p=P, c=C)

    with tile.TileContext(nc) as tc:
        with (
            tc.tile_pool(name="xs", bufs=3) as xs_pool,
            tc.tile_pool(name="es", bufs=3) as es_pool,
            tc.tile_pool(name="zs", bufs=3) as zs_pool,
            tc.tile_pool(name="stats", bufs=12) as st_pool,
        ):
            for n in range(N_BLOCKS):
                xt = xs_pool.tile([P, C, DEPTH], F32)
                nc.sync.dma_start(out=xt[:], in_=xv[:, n, :, :])
                et = es_pool.tile([P, C, DEPTH], F32)
                zt = zs_pool.tile([P, C, DEPTH], F32)
                for c in range(C):
                    v8 = st_pool.tile([P, 8], F32)
                    e8 = st_pool.tile([P, 8], F32)
                    s = st_pool.tile([P, 1], F32)
                    r = st_pool.tile([P, 1], F32)
                    nc.vector.max(out=v8[:], in_=xt[:, c, :])
                    nc.scalar.activation(out=et[:, c, :], in_=xt[:, c, :], func=Exp)
                    nc.scalar.activation(out=e8[:], in_=v8[:], func=Exp, accum_out=s[:])
                    nc.vector.reciprocal(out=r[:], in_=s[:])
                    nc.vector.match_replace(
                        out=zt[:, c, :],
                        in_to_replace=e8[:],
                        in_values=et[:, c, :],
                        imm_value=0.0,
                    )
                    nc.vector.tensor_sub(
                        out=zt[:, c, :], in0=et[:, c, :], in1=zt[:, c, :]
                    )
                    nc.gpsimd.tensor_scalar(
                        zt[:, c, :], zt[:, c, :], r[:], None, mybir.AluOpType.mult
                    )
                nc.sync.dma_start(out=ov[:, n, :, :], in_=zt[:])
    nc.compile()
    return nc


def kernel(**inputs: np.ndarray) -> np.ndarray:
    full = np.ascontiguousarray(inputs["inputs"], dtype=np.float32)
    assert full.shape == (BATCH, DEPTH), full.shape

    nc = _build()
    in_maps = [
        {"x": np.ascontiguousarray(full[i * ROWS_PER_CORE : (i + 1) * ROWS_PER_CORE])}
        for i in range(N_CORES)
    ]
    trace = bool(os.environ.get("BASS_TRACE"))
    if trace:
        trace = _install_ntff_hook()
    try:
        res = run_bass_kernel_spmd(
            nc, in_maps, core_ids=list(range(N_CORES)), trace=trace
        )
    except Exception:
        if not trace:
            raise
        res = run_bass_kernel_spmd(
            nc, in_maps, core_ids=list(range(N_CORES)), trace=False
        )
    kernel.last_result = res
    return np.concatenate([r["out"] for r in res.results], axis=0)


# revision 7
# speedup vs baseline: 3.5757x; 3.5757x over previous
"""Trainium2 Bass kernel for AnnealingTopKSoftMax (top-8 masked softmax).

Computes, for each row of a [131072, 512] f32 tensor:
  out = softmax(where(mask_top8(x), x, -1e16))
which equals: exp(x)/sum(exp(top8(x))) at the top-8 positions, 0 elsewhere.

Strategy (pure data parallelism, batch axis sharded over 8 NeuronCores).
Per [128, 8, 512] block (rows on partitions, 8 row-subtiles per partition):
  per subtile c:
    v8[c] = max8(x)                      # DVE: 8 largest per row (desc)
    y     = match_replace(x, v8[c], +2e38)  # DVE: mark EXACTLY the top-8
  e8 = exp(v8_all)                       # ACT tiny [128, 64]
  s8 = reduce_add(e8 over last axis)     # DVE tiny -> per-subtile denom
  r8 = 1/s8 ; lnr8 = ln(r8)              # DVE + ACT tiny [128, 8]
  per subtile c:
    e = exp(x + lnr8[c])                 # ACT: exp(x)/s, per-partition bias
  out = (y > 1e38) * e                   # DVE: ONE batched [128, 4096] pass
match_replace replaces exactly one occurrence per needle (first match),
reproducing jax.lax.top_k's lowest-index tie-breaking exactly. exp never
overflows (|x| <= ~6 for this problem's N(0,1) data).
"""

import os
import sys
import types

import numpy as np

import concourse.bacc as bacc
import concourse.tile as tile
from concourse import mybir
from concourse.bass_utils import run_bass_kernel_spmd


def _install_ntff_hook() -> bool:
    """Provide antenv.axon_hooks (absent in this container) so
    run_bass_kernel_spmd(trace=True) can capture NTFF profiles under axon.
    Mirrors trn_agent_boot's registration. Returns False when unavailable."""
    try:
        from antenv.axon_hooks import get_axon_ntff_profile_hook  # noqa: F401

        return True
    except ImportError:
        pass
    try:
        import antenv
        from trn_agent_boot.trn_boot import _ntff_profile_via_ctypes

        hook = _ntff_profile_via_ctypes("/opt/axon/libaxon_pjrt.so")
        mod = types.ModuleType("antenv.axon_hooks")
        _h = [hook]
        mod.set_axon_ntff_profile_hook = lambda h: _h.__setitem__(0, h)
        mod.get_axon_ntff_profile_hook = lambda: _h[0]
        sys.modules["antenv.axon_hooks"] = mod
        antenv.axon_hooks = mod
        return hook is not None
    except Exception:
        return False


N_CORES = 8
BATCH = 131072
DEPTH = 512
ROWS_PER_CORE = BATCH // N_CORES  # 16384
P = 128          # SBUF partitions; rows per sub-tile
C = 8            # row-subtiles per partition per block (16KB contiguous DMA)
BLOCK_ROWS = P * C               # 1024
N_BLOCKS = ROWS_PER_CORE // BLOCK_ROWS  # 16

F32 = mybir.dt.float32
Exp = mybir.ActivationFunctionType.Exp
Ln = mybir.ActivationFunctionType.Ln

MARK = 2.0e38    # match_replace marker for selected positions
THRESH = 1.0e38  # (y > THRESH) <=> position was selected


def _build(n_blocks: int = N_BLOCKS):
    rows = n_blocks * BLOCK_ROWS
    nc = bacc.Bacc(
        "TRN2", target_bir_lowering=False, debug=False, num_devices=N_CORES
    )
    x = nc.dram_tensor("x", [rows, DEPTH], F32, kind="ExternalInput")
    out = nc.dram_tensor("out", [rows, DEPTH], F32, kind="ExternalOutput")

    # row = n*1024 + p*8 + c  ->  partition p holds 8 consecutive rows per block
    xv = x.ap().rearrange("(n p c) d -> p n c d", p=P, c=C)
    ov = out.ap().rearrange("(n p c) d -> p n c d", p=P, c=C)

    with tile.TileContext(nc) as tc:
        with (
            tc.tile_pool(name="xs", bufs=3) as xs_pool,
            tc.tile_pool(name="es", bufs=3) as es_pool,
            tc.tile_pool(name="ys", bufs=3) as ys_pool,
            tc.tile_pool(name="stats", bufs=4) as st_pool,
        ):
            for n in range(n_blocks):
                xt = xs_pool.tile([P, C, DEPTH], F32)
                nc.sync.dma_start(out=xt[:], in_=xv[:, n, :, :])
                et = es_pool.tile([P, C, DEPTH], F32)
                yt = ys_pool.tile([P, C, DEPTH], F32)
                v8 = st_pool.tile([P, C, 8], F32)
                e8 = st_pool.tile([P, C, 8], F32)
                s8 = st_pool.tile([P, C], F32)
                r8 = st_pool.tile([P, C], F32)
                lnr8 = st_pool.tile([P, C], F32)
                for c in range(C):
                    nc.vector.max(out=v8[:, c, :], in_=xt[:, c, :])
                    nc.vector.match_replace(
                        out=yt[:, c, :],
                        in_to_replace=v8[:, c, :],
                        in_values=xt[:, c, :],
                        imm_value=MARK,
                    )
                nc.scalar.activation(
                    out=e8.rearrange("p c k -> p (c k)"),
                    in_=v8.rearrange("p c k -> p (c k)"),
                    func=Exp,
                )
                nc.vector.tensor_reduce(
                    out=s8[:],
                    in_=e8[:],
                    axis=mybir.AxisListType.X,
                    op=mybir.AluOpType.add,
                )
                nc.vector.reciprocal(out=r8[:], in_=s8[:])
                nc.scalar.activation(out=lnr8[:], in_=r8[:], func=Ln)
                for c in range(C):
                    nc.scalar.activation(
                        out=et[:, c, :],
                        in_=xt[:, c, :],
                        func=Exp,
                        bias=lnr8[:, c : c + 1],
                    )
                # one batched pass over the whole block: (y > 1e38) * e
                nc.vector.scalar_tensor_tensor(
                    out=et[:],
                    in0=yt[:],
                    scalar=THRESH,
                    in1=et[:],
                    op0=mybir.AluOpType.is_gt,
                    op1=mybir.AluOpType.mult,
                )
                nc.sync.dma_start(out=ov[:, n, :, :], in_=et[:])
    nc.compile()
    return nc


def kernel(**inputs: np.ndarray) -> np.ndarray:
    full = np.ascontiguousarray(inputs["inputs"], dtype=np.float32)
    assert full.shape == (BATCH, DEPTH), full.shape

    nc = _build()
    in_maps = [
        {"x": np.ascontiguousarray(full[i * ROWS_PER_CORE : (i + 1) * ROWS_PER_CORE])}
        for i in range(N_CORES)
    ]
    trace = bool(os.environ.get("BASS_TRACE"))
    if trace:
        trace = _install_ntff_hook()
    try:
        res = run_bass_kernel_spmd(
            nc, in_maps, core_ids=list(range(N_CORES)), trace=trace
        )
    except Exception:
        if not trace:
            raise
        res = run_bass_kernel_spmd(
            nc, in_maps, core_ids=list(range(N_CORES)), trace=False
        )
    kernel.last_result = res
    return np.concatenate([r["out"] for r in res.results], axis=0)
